# revision 5
# baseline (speedup 1.0000x reference)
"""Binarized complex-style dense layer on 8 TRN2 NeuronCores.

Computes out = sign(x + eps) @ K^T with K = [[br, -bi], [bi, br]],
br = sign(weight_real + eps), bi = sign(weight_imag + eps).

Sharding: data-parallel over the batch dim (131072 rows -> 16384 per core),
weights replicated. Forward only, so no collectives.

Per-core pipeline (all values +-1 so bf16 matmul is exact; sums <= 256 are
exact in fp32 PSUM):
  DMA x chunk [128, 16, 256] f32 -> SBUF
  PE  transpose 128x128 f32 sub-tiles -> PSUM (k on partitions)
  ACT sign(v + eps) PSUM f32 -> SBUF bf16   (binarize fused into the copy)
  PE  matmul xbT[k,b] @ kernelT[k,o] -> PSUM f32 [b, o]
  DVE copy PSUM -> SBUF f32
  DMA out chunk -> DRAM
"""

import numpy as np

N_CORES = 8
B_TOTAL = 131072
ROWS_PER_CORE = B_TOTAL // N_CORES  # 16384
FAN = 128
K2 = 2 * FAN  # 256 = 2*fan_in = 2*fan_out
CHUNK_ROWS = 2048  # rows per DMA (2 MB)
EPS = 1e-6

_NC_CACHE = {}


def _build_nc(rows_per_core):
    from concourse import bacc, masks, mybir, tile

    f32 = mybir.dt.float32
    bf16 = mybir.dt.bfloat16
    Sign = mybir.ActivationFunctionType.Sign

    chunk_rows = min(CHUNK_ROWS, rows_per_core)
    n_chunks = rows_per_core // chunk_rows
    assert n_chunks * chunk_rows == rows_per_core
    n_j = chunk_rows // 128  # 128-row sub-tiles per chunk
    assert n_j % 2 == 0, "sub-tiles are processed in pairs"

    nc = bacc.Bacc("TRN2", target_bir_lowering=False, debug=False)

    x_d = nc.dram_tensor("x", [rows_per_core, K2], f32, kind="ExternalInput")
    wr_d = nc.dram_tensor("weight_real", [FAN, FAN], f32, kind="ExternalInput")
    wi_d = nc.dram_tensor("weight_imag", [FAN, FAN], f32, kind="ExternalInput")
    out_d = nc.dram_tensor("out", [rows_per_core, K2], f32, kind="ExternalOutput")

    # DRAM views: chunk c, partition p holds row c*chunk_rows + j*128 + p.
    x_v = x_d[:].rearrange("(c j p) k -> c p j k", p=128, j=n_j)
    out_v = out_d[:].rearrange("(c j p) k -> c p j k", p=128, j=n_j)

    with tile.TileContext(nc) as tc:
        with (
            tc.tile_pool(name="const", bufs=1) as const_pool,
            tc.tile_pool(name="kt", bufs=1) as kt_pool,
            tc.tile_pool(name="xin", bufs=3) as x_pool,
            tc.tile_pool(name="oout", bufs=3) as o_pool,
            tc.tile_pool(name="xbt", bufs=4) as xbt_pool,
            tc.tile_pool(name="pwt", bufs=1, space="PSUM") as wt_pool,
            tc.tile_pool(name="ptp", bufs=3, space="PSUM") as tp_pool,
            tc.tile_pool(name="pout", bufs=3, space="PSUM") as po_pool,
        ):
            ident = const_pool.tile([128, 128], f32)
            masks.make_identity(nc, ident[:])
            eps_pos = const_pool.tile([128, 1], f32)
            nc.gpsimd.memset(eps_pos[:], EPS)
            eps_neg = const_pool.tile([128, 1], f32)
            nc.gpsimd.memset(eps_neg[:], -EPS)

            # Build kernelT [256 k, 256 o] as two [128, 256] bf16 tiles:
            #   kT0 = [ sign(wr^T) | sign(wi^T) ]   (k in [0,128))
            #   kT1 = [ -sign(wi^T) | sign(wr^T) ]  (k in [128,256))
            w_sb = const_pool.tile([128, 256], f32)
            nc.sync.dma_start(out=w_sb[:, 0:128], in_=wr_d[:])
            nc.sync.dma_start(out=w_sb[:, 128:256], in_=wi_d[:])
            wt_ps = wt_pool.tile([128, 256], f32)
            nc.tensor.transpose(wt_ps[:, 0:128], w_sb[:, 0:128], ident[:])
            nc.tensor.transpose(wt_ps[:, 128:256], w_sb[:, 128:256], ident[:])
            kt0 = kt_pool.tile([128, 256], bf16)
            kt1 = kt_pool.tile([128, 256], bf16)
            nc.scalar.activation(kt0[:, 0:128], wt_ps[:, 0:128], Sign, bias=eps_pos[:])
            nc.scalar.activation(kt0[:, 128:256], wt_ps[:, 128:256], Sign, bias=eps_pos[:])
            nc.scalar.activation(
                kt1[:, 0:128], wt_ps[:, 128:256], Sign, bias=eps_neg[:], scale=-1.0
            )
            nc.scalar.activation(kt1[:, 128:256], wt_ps[:, 0:128], Sign, bias=eps_pos[:])

            for c in range(n_chunks):
                xt = x_pool.tile([128, n_j * 256], f32, tag="xt")
                nc.sync.dma_start(
                    out=xt[:].rearrange("p (j k) -> p j k", k=256), in_=x_v[c]
                )
                ot = o_pool.tile([128, n_j * 256], f32, tag="ot")
                for jj in range(n_j // 2):
                    # Two 128-row sub-tiles share one PSUM bank so the
                    # ACT/DVE fixed overhead amortizes over 512 columns.
                    tp = tp_pool.tile([128, 512], f32, tag="tp")
                    for h in range(2):
                        j = 2 * jj + h
                        nc.tensor.transpose(
                            tp[:, h * 256 : h * 256 + 128],
                            xt[:, j * 256 : j * 256 + 128],
                            ident[:],
                        )
                        nc.tensor.transpose(
                            tp[:, h * 256 + 128 : h * 256 + 256],
                            xt[:, j * 256 + 128 : j * 256 + 256],
                            ident[:],
                        )
                    xbt = xbt_pool.tile([128, 512], bf16, tag="xbt")
                    nc.scalar.activation(xbt[:], tp[:], Sign, bias=eps_pos[:])
                    po = po_pool.tile([128, 512], f32, tag="po")
                    for h in range(2):
                        nc.tensor.matmul(
                            po[:, h * 256 : h * 256 + 256],
                            xbt[:, h * 256 : h * 256 + 128],
                            kt0[:],
                            start=True,
                            stop=False,
                        )
                        nc.tensor.matmul(
                            po[:, h * 256 : h * 256 + 256],
                            xbt[:, h * 256 + 128 : h * 256 + 256],
                            kt1[:],
                            start=False,
                            stop=True,
                        )
                    nc.vector.tensor_copy(ot[:, jj * 512 : (jj + 1) * 512], po[:])
                nc.sync.dma_start(
                    out=out_v[c], in_=ot[:].rearrange("p (j k) -> p j k", k=256)
                )

    nc.compile()
    return nc


def get_nc(rows_per_core=ROWS_PER_CORE):
    if rows_per_core not in _NC_CACHE:
        _NC_CACHE[rows_per_core] = _build_nc(rows_per_core)
    return _NC_CACHE[rows_per_core]


def kernel(x, weight_real, weight_imag, trace=False, tmpdir=None):
    from concourse import bass_utils

    x = np.ascontiguousarray(np.asarray(x, dtype=np.float32))
    wr = np.ascontiguousarray(np.asarray(weight_real, dtype=np.float32))
    wi = np.ascontiguousarray(np.asarray(weight_imag, dtype=np.float32))
    assert x.shape == (B_TOTAL, K2) and wr.shape == (FAN, FAN) and wi.shape == (FAN, FAN)

    nc = get_nc()
    in_maps = [
        {
            "x": x[i * ROWS_PER_CORE : (i + 1) * ROWS_PER_CORE],
            "weight_real": wr,
            "weight_imag": wi,
        }
        for i in range(N_CORES)
    ]
    res = bass_utils.run_bass_kernel_spmd(
        nc, in_maps, core_ids=list(range(N_CORES)), trace=trace, tmpdir=tmpdir
    )
    out = np.concatenate([res.results[i]["out"] for i in range(N_CORES)], axis=0)
    if trace:
        return out, res
    return out


# revision 9
# speedup vs baseline: 1.0824x; 1.0824x over previous
"""Binarized complex-style dense layer on 8 TRN2 NeuronCores.

Computes out = sign(x + eps) @ K^T with K = [[br, -bi], [bi, br]],
br = sign(weight_real + eps), bi = sign(weight_imag + eps).

Sharding: data-parallel over the batch dim (131072 rows -> 16384 per core),
weights replicated. Forward only, so no collectives.

Per-core pipeline (all values +-1 so bf16 matmul is exact; sums <= 256 are
exact in fp32 PSUM):
  DMA x chunk [128, 16, 256] f32 -> SBUF
  PE  transpose 128x128 f32 sub-tiles -> PSUM (k on partitions)
  ACT sign(v + eps) PSUM f32 -> SBUF bf16   (binarize fused into the copy)
  PE  matmul xbT[k,b] @ kernelT[k,o] -> PSUM f32 [b, o]
  DVE copy PSUM -> SBUF f32
  DMA out chunk -> DRAM
"""

import numpy as np

N_CORES = 8
B_TOTAL = 131072
ROWS_PER_CORE = B_TOTAL // N_CORES  # 16384
FAN = 128
K2 = 2 * FAN  # 256 = 2*fan_in = 2*fan_out
CHUNK_ROWS = 1024  # rows per DMA (1 MB)
EPS = 1e-6

_NC_CACHE = {}


def _build_nc(rows_per_core):
    from concourse import bacc, masks, mybir, tile

    f32 = mybir.dt.float32
    bf16 = mybir.dt.bfloat16
    Sign = mybir.ActivationFunctionType.Sign

    chunk_rows = min(CHUNK_ROWS, rows_per_core)
    n_chunks = rows_per_core // chunk_rows
    assert n_chunks * chunk_rows == rows_per_core
    n_j = chunk_rows // 128  # 128-row sub-tiles per chunk
    assert n_j % 2 == 0, "sub-tiles are processed in pairs"

    nc = bacc.Bacc("TRN2", target_bir_lowering=False, debug=False)

    x_d = nc.dram_tensor("x", [rows_per_core, K2], f32, kind="ExternalInput")
    wr_d = nc.dram_tensor("weight_real", [FAN, FAN], f32, kind="ExternalInput")
    wi_d = nc.dram_tensor("weight_imag", [FAN, FAN], f32, kind="ExternalInput")
    out_d = nc.dram_tensor("out", [rows_per_core, K2], f32, kind="ExternalOutput")

    # DRAM views: chunk c, partition p holds rows c*chunk_rows + p*n_j + r
    # (r = 0..n_j-1), i.e. each partition reads/writes one contiguous
    # n_j*1KB run per chunk -> large DMA descriptors at line rate.
    x_v = x_d[:].rearrange("(c p r) k -> c p (r k)", p=128, r=n_j)
    out_v = out_d[:].rearrange("(c p r) k -> c p (r k)", p=128, r=n_j)

    with tile.TileContext(nc) as tc:
        with (
            tc.tile_pool(name="const", bufs=1) as const_pool,
            tc.tile_pool(name="kt", bufs=1) as kt_pool,
            tc.tile_pool(name="xin", bufs=3) as x_pool,
            tc.tile_pool(name="oout", bufs=3) as o_pool,
            tc.tile_pool(name="xbt", bufs=4) as xbt_pool,
            tc.tile_pool(name="pwt", bufs=1, space="PSUM") as wt_pool,
            tc.tile_pool(name="ptp", bufs=3, space="PSUM") as tp_pool,
            tc.tile_pool(name="pout", bufs=3, space="PSUM") as po_pool,
        ):
            ident = const_pool.tile([128, 128], f32)
            masks.make_identity(nc, ident[:])
            eps_pos = const_pool.tile([128, 1], f32)
            nc.gpsimd.memset(eps_pos[:], EPS)
            eps_neg = const_pool.tile([128, 1], f32)
            nc.gpsimd.memset(eps_neg[:], -EPS)

            # Build kernelT [256 k, 256 o] as two [128, 256] bf16 tiles:
            #   kT0 = [ sign(wr^T) | sign(wi^T) ]   (k in [0,128))
            #   kT1 = [ -sign(wi^T) | sign(wr^T) ]  (k in [128,256))
            w_sb = const_pool.tile([128, 256], f32)
            nc.sync.dma_start(out=w_sb[:, 0:128], in_=wr_d[:])
            nc.sync.dma_start(out=w_sb[:, 128:256], in_=wi_d[:])
            wt_ps = wt_pool.tile([128, 256], f32)
            nc.tensor.transpose(wt_ps[:, 0:128], w_sb[:, 0:128], ident[:])
            nc.tensor.transpose(wt_ps[:, 128:256], w_sb[:, 128:256], ident[:])
            kt0 = kt_pool.tile([128, 256], bf16)
            kt1 = kt_pool.tile([128, 256], bf16)
            nc.scalar.activation(kt0[:, 0:128], wt_ps[:, 0:128], Sign, bias=eps_pos[:])
            nc.scalar.activation(kt0[:, 128:256], wt_ps[:, 128:256], Sign, bias=eps_pos[:])
            nc.scalar.activation(
                kt1[:, 0:128], wt_ps[:, 128:256], Sign, bias=eps_neg[:], scale=-1.0
            )
            nc.scalar.activation(kt1[:, 128:256], wt_ps[:, 0:128], Sign, bias=eps_pos[:])

            for c in range(n_chunks):
                xt = x_pool.tile([128, n_j * 256], f32, tag="xt")
                nc.sync.dma_start(out=xt[:], in_=x_v[c])
                ot = o_pool.tile([128, n_j * 256], f32, tag="ot")
                for jj in range(n_j // 2):
                    # Two 128-row sub-tiles share one PSUM bank so the
                    # ACT/DVE fixed overhead amortizes over 512 columns.
                    tp = tp_pool.tile([128, 512], f32, tag="tp")
                    for h in range(2):
                        j = 2 * jj + h
                        nc.tensor.transpose(
                            tp[:, h * 256 : h * 256 + 128],
                            xt[:, j * 256 : j * 256 + 128],
                            ident[:],
                        )
                        nc.tensor.transpose(
                            tp[:, h * 256 + 128 : h * 256 + 256],
                            xt[:, j * 256 + 128 : j * 256 + 256],
                            ident[:],
                        )
                    xbt = xbt_pool.tile([128, 512], bf16, tag="xbt")
                    nc.scalar.activation(xbt[:], tp[:], Sign, bias=eps_pos[:])
                    po = po_pool.tile([128, 512], f32, tag="po")
                    for h in range(2):
                        nc.tensor.matmul(
                            po[:, h * 256 : h * 256 + 256],
                            xbt[:, h * 256 : h * 256 + 128],
                            kt0[:],
                            start=True,
                            stop=False,
                        )
                        nc.tensor.matmul(
                            po[:, h * 256 : h * 256 + 256],
                            xbt[:, h * 256 + 128 : h * 256 + 256],
                            kt1[:],
                            start=False,
                            stop=True,
                        )
                    nc.vector.tensor_copy(ot[:, jj * 512 : (jj + 1) * 512], po[:])
                nc.sync.dma_start(out=out_v[c], in_=ot[:])

    nc.compile()
    return nc


def get_nc(rows_per_core=ROWS_PER_CORE):
    if rows_per_core not in _NC_CACHE:
        _NC_CACHE[rows_per_core] = _build_nc(rows_per_core)
    return _NC_CACHE[rows_per_core]


def kernel(x, weight_real, weight_imag, trace=False, tmpdir=None):
    from concourse import bass_utils

    x = np.ascontiguousarray(np.asarray(x, dtype=np.float32))
    wr = np.ascontiguousarray(np.asarray(weight_real, dtype=np.float32))
    wi = np.ascontiguousarray(np.asarray(weight_imag, dtype=np.float32))
    assert x.shape == (B_TOTAL, K2) and wr.shape == (FAN, FAN) and wi.shape == (FAN, FAN)

    nc = get_nc()
    in_maps = [
        {
            "x": x[i * ROWS_PER_CORE : (i + 1) * ROWS_PER_CORE],
            "weight_real": wr,
            "weight_imag": wi,
        }
        for i in range(N_CORES)
    ]
    res = bass_utils.run_bass_kernel_spmd(
        nc, in_maps, core_ids=list(range(N_CORES)), trace=trace, tmpdir=tmpdir
    )
    out = np.concatenate([res.results[i]["out"] for i in range(N_CORES)], axis=0)
    if trace:
        return out, res
    return out
